# revision 11
# baseline (speedup 1.0000x reference)
import numpy as np
import ml_dtypes

import concourse.bass as bass
import concourse.bacc as bacc
import concourse.mybir as mybir
import concourse.tile as tile
from concourse import bass_utils

F32 = mybir.dt.float32
BF16 = mybir.dt.bfloat16
AF = mybir.ActivationFunctionType
ALU = mybir.AluOpType

B, N, HID = 4, 4096, 128
HALF = N // 2           # own output rows per core
NB = N // 128           # 32 j blocks
HB = HALF // 128        # 16 own i blocks
NGROUP = 8              # stream groups (4 j-blocks each)
NITER = 8               # Jacobi LSTM matmul iterations (plus a zeroth)

_CACHED = {}


def build_nc(dbg=False):
    nc = bacc.Bacc("TRN2", target_bir_lowering=False, debug=False, num_devices=8)
    if dbg:
        ddeg = nc.dram_tensor("ddeg", [128, 16], F32, kind="ExternalOutput")
        ddinv = nc.dram_tensor("ddinv", [128, NB], F32, kind="ExternalOutput")
        dlw = nc.dram_tensor("dlw", [128, 128], F32, kind="ExternalOutput")
        dzx = nc.dram_tensor("dzx", [128, 512], F32, kind="ExternalOutput")
        dagg = nc.dram_tensor("dagg", [128, 512], F32, kind="ExternalOutput")
        dxs = nc.dram_tensor("dxs", [128, 256], F32, kind="ExternalOutput")

    # A_hat^T slice: [4096 j, 2048 own-i] bf16, identity pre-added on host
    atf = nc.dram_tensor("atf", [N, HALF], BF16, kind="ExternalInput")
    xf = nc.dram_tensor("xf", [N, HID], F32, kind="ExternalInput")
    xdf = nc.dram_tensor("xdf", [128, 128], F32, kind="ExternalInput")
    cwt = nc.dram_tensor("cwt", [3, 128, 128], F32, kind="ExternalInput")
    wihT = nc.dram_tensor("wihT", [128, 512], F32, kind="ExternalInput")
    whhT = nc.dram_tensor("whhT", [128, 512], BF16, kind="ExternalInput")
    wihn = nc.dram_tensor("wihn", [512, 128], F32, kind="ExternalInput")
    bias4 = nc.dram_tensor("bias4", [128, 4], F32, kind="ExternalInput")
    cbb = nc.dram_tensor("cbb", [128, 128], F32, kind="ExternalInput")
    eyeb = nc.dram_tensor("eyeb", [128, 128], BF16, kind="ExternalInput")
    mlo = nc.dram_tensor("mlo", [128, 1], F32, kind="ExternalInput")
    mhi = nc.dram_tensor("mhi", [128, 1], F32, kind="ExternalInput")
    out = nc.dram_tensor("out", [HALF, HID], F32, kind="ExternalOutput")

    with tile.TileContext(nc) as tc:
        with (
            tc.tile_pool(name="const", bufs=1) as cp,
            tc.tile_pool(name="big", bufs=1) as bigp,
            tc.tile_pool(name="at", bufs=NGROUP) as atp,
            tc.tile_pool(name="work", bufs=2) as wkp,
            tc.tile_pool(name="ps1", bufs=2, space="PSUM") as ps1,
            tc.tile_pool(name="psdeg", bufs=1, space="PSUM") as psdeg,
            tc.tile_pool(name="psagg", bufs=4, space="PSUM") as psagg,
            tc.tile_pool(name="dram", bufs=1, space="DRAM") as dram,
        ):
            # ---------- small constant loads ----------
            wihT_sb = cp.tile([128, 512], F32, tag="wihT")
            whhT_sb = cp.tile([128, 512], BF16, tag="whhT")
            bias4_sb = cp.tile([128, 4], F32, tag="bias4")
            cbb_sb = cp.tile([128, 128], F32, tag="cbb")
            eyeb_sb = cp.tile([128, 128], BF16, tag="eyeb")
            mlo_sb = cp.tile([128, 1], F32, tag="mlo")
            mhi_sb = cp.tile([128, 1], F32, tag="mhi")
            xdf_sb = cp.tile([128, 128], F32, tag="xdf")
            cwt_sb = cp.tile([128, 384], F32, tag="cwt")
            nc.sync.dma_start(wihT_sb[:], wihT[:])
            nc.sync.dma_start(whhT_sb[:], whhT[:])
            nc.sync.dma_start(bias4_sb[:], bias4[:])
            nc.sync.dma_start(cbb_sb[:], cbb[:])
            nc.sync.dma_start(eyeb_sb[:], eyeb[:])
            nc.sync.dma_start(mlo_sb[:], mlo[:])
            nc.sync.dma_start(mhi_sb[:], mhi[:])
            nc.sync.dma_start(xdf_sb[:], xdf[:])
            for k in range(3):
                nc.sync.dma_start(cwt_sb[:, k * 128:(k + 1) * 128], cwt[k])
            wn_sb = cp.tile([128, 512], F32, tag="wn")
            nc.sync.dma_start(
                wn_sb[:].rearrange("p (g d) -> p g d", g=4),
                wihn[:].rearrange("(g p) d -> p g d", p=128),
            )

            ones_sb = cp.tile([128, 1], BF16, tag="ones")
            nc.vector.memset(ones_sb[:], 1.0)

            # ---------- stream A_hat^T in 8 groups of 4 j-blocks ----------
            at_g = []
            for g in range(NGROUP):
                t = atp.tile([128, 4 * HALF], BF16, tag="at")
                nc.sync.dma_start(
                    t[:].rearrange("p (t c) -> p t c", t=4),
                    atf[g * 512:(g + 1) * 512, :].rearrange("(t p) c -> p t c", p=128),
                )
                at_g.append(t)

            # xf after the stream issue (needed later)
            xf_sb = bigp.tile([128, N], F32, tag="xfsb")
            nc.sync.dma_start(
                xf_sb[:].rearrange("p (n d) -> p n d", n=NB),
                xf[:].rearrange("(n p) d -> p n d", p=128),
            )

            # ---------- conv -> dynT ----------
            dfpad = cp.tile([128, 130], F32, tag="dfpad")
            nc.vector.memset(dfpad[:], 0.0)
            nc.vector.tensor_copy(dfpad[:, 1:129], xdf_sb[:])
            dyn_ps = ps1.tile([128, 512], F32, tag="a")
            for k in range(3):
                nc.tensor.matmul(
                    dyn_ps[:, 0:128], dfpad[:, k:k + 128],
                    cwt_sb[:, k * 128:(k + 1) * 128],
                    start=(k == 0), stop=(k == 2),
                )
            dynT_sb = cp.tile([128, 128], F32, tag="dynT")
            nc.vector.tensor_copy(dynT_sb[:], dyn_ps[:, 0:128])

            # S[u, g] = sum_f w_ih_perm[g*128+u, f]
            S_sb = cp.tile([128, 4], F32, tag="S")
            for g4 in range(4):
                nc.vector.reduce_sum(
                    S_sb[:, g4:g4 + 1], wn_sb[:, g4 * 128:(g4 + 1) * 128],
                    axis=mybir.AxisListType.X,
                )

            # ---------- Zx_sb[u, g*128+t] ----------
            zx_ps = ps1.tile([128, 512], F32, tag="a")
            for g4 in range(4):
                nc.tensor.matmul(
                    zx_ps[:, g4 * 128:(g4 + 1) * 128],
                    wihT_sb[:, g4 * 128:(g4 + 1) * 128], dynT_sb[:],
                    start=True, stop=True, skip_group_check=True,
                )
            Zx_sb = bigp.tile([128, 512], F32, tag="Zx")
            for g4 in range(4):
                sl = slice(g4 * 128, (g4 + 1) * 128)
                nc.vector.tensor_scalar_add(
                    Zx_sb[:, sl], zx_ps[:, sl], bias4_sb[:, g4:g4 + 1])
                corr = wkp.tile([128, 128], F32, tag="corr")
                nc.vector.tensor_scalar_mul(corr[:], cbb_sb[:], S_sb[:, g4:g4 + 1])
                nc.vector.tensor_tensor(Zx_sb[:, sl], Zx_sb[:, sl], corr[:], op=ALU.add)

            # ---------- Jacobi LSTM ----------
            hq = cp.tile([128, 129], BF16, tag="hq")   # col0 = 0, cols 1..128 = h_t
            nc.vector.memset(hq[:], 0.0)

            zps = []
            for it in range(NITER + 1):
                zps.append(ps1.tile([128, 512], F32, tag="a", name=f"zp{it}"))

            def lstm_iter(zp):
                ga = wkp.tile([128, 384], F32, tag="ga")
                tg = wkp.tile([128, 128], F32, tag="tg")
                ig = wkp.tile([128, 128], F32, tag="ig")
                cc = wkp.tile([128, 128], F32, tag="cc")
                tcn = wkp.tile([128, 128], F32, tag="tc")
                nc.scalar.activation(ga[:], zp[:, 0:384], AF.Sigmoid)
                nc.scalar.activation(tg[:], zp[:, 384:512], AF.Tanh)
                nc.vector.tensor_tensor(ig[:], ga[:, 0:128], tg[:], op=ALU.mult)
                nc.vector.tensor_tensor_scan(
                    cc[:], ga[:, 128:256], ig[:], 0.0, ALU.mult, ALU.add)
                nc.scalar.activation(tcn[:], cc[:], AF.Tanh)
                nc.vector.tensor_tensor(hq[:, 1:129], ga[:, 256:384], tcn[:], op=ALU.mult)

            # zeroth iteration: gates from Zx only (h = 0)
            nc.vector.tensor_copy(zps[0][:], Zx_sb[:])
            nc.vector.tensor_copy(zps[1][:], Zx_sb[:])
            lstm_iter(zps[0])

            # ---------- deg matmuls interleaved with LSTM iterations ----------
            deg_ps = psdeg.tile([128, 512], F32, tag="deg")
            deg_ps2 = psdeg.tile([128, 512], F32, tag="deg2")

            def deg_slot(k):
                return deg_ps[32 * k:32 * k + 1, :] if k < 3 else deg_ps2[0:1, :]

            def deg_group(g):
                for jj in range(4):
                    jb = g * 4 + jj
                    for k in range(4):
                        nc.tensor.matmul(
                            deg_slot(k),
                            ones_sb[:],
                            at_g[g][:, jj * HALF + k * 512: jj * HALF + (k + 1) * 512],
                            start=(jb == 0), stop=(jb == NB - 1),
                            skip_group_check=True,
                        )

            def full_iter(it):
                # matmuls accumulate onto preloaded Zx in psum bank it%? (ring)
                zp = zps[it]
                for g4 in range(4):
                    nc.tensor.matmul(
                        zp[:, g4 * 128:(g4 + 1) * 128],
                        whhT_sb[:, g4 * 128:(g4 + 1) * 128], hq[:, 0:128],
                        start=False, stop=True, skip_group_check=True,
                    )
                lstm_iter(zp)
                if it + 1 <= NITER:
                    nc.vector.tensor_copy(zps[it + 1][:], Zx_sb[:])

            # interleave: groups 0..4 with iters 1..5, then groups 5..7, then iters 6..8
            for g in range(5):
                deg_group(g)
                full_iter(g + 1)
            for g in range(5, NGROUP):
                deg_group(g)
            for it in range(6, NITER + 1):
                full_iter(it)

            # lw[t, u] = h_t[u] via plain matmul with identity moving
            lw_ps = ps1.tile([128, 512], F32, tag="a")
            nc.tensor.matmul(
                lw_ps[:, 0:128], hq[:, 1:129], eyeb_sb[:],
                start=True, stop=True, skip_group_check=True,
            )
            lw_sb = cp.tile([128, 128], F32, tag="lw")
            nc.vector.tensor_copy(lw_sb[:], lw_ps[:, 0:128])

            # ---------- deg reshape [1,512]x4 -> [128,16], AllGather ----------
            deg_row = cp.tile([1, 2048], F32, tag="degrow")
            for k in range(4):
                nc.vector.tensor_copy(deg_row[:, k * 512:(k + 1) * 512], deg_slot(k))
            deg_lin = dram.tile([1, 2048], F32)
            nc.sync.dma_start(deg_lin[:], deg_row[:])
            deg_sb = cp.tile([128, 16], F32, tag="degsb")
            nc.sync.dma_start(
                deg_sb[:],
                deg_lin[:].rearrange("o (b p) -> (o p) b", p=128),
            )
            cc_in = dram.tile([128, 16], F32)
            cc_out = dram.tile([2, 128, 16], F32)
            nc.gpsimd.dma_start(cc_in[:], deg_sb[:])
            nc.gpsimd.collective_compute(
                "AllGather", ALU.bypass,
                replica_groups=[[0, 1], [2, 3], [4, 5], [6, 7]],
                ins=[cc_in.opt()], outs=[cc_out.opt()],
            )
            deg_all = cp.tile([128, NB], F32, tag="dega")
            nc.sync.dma_start(deg_all[:, 0:16], cc_out[0])
            nc.sync.dma_start(deg_all[:, 16:32], cc_out[1])
            sq = cp.tile([128, NB], F32, tag="sq")
            nc.scalar.activation(sq[:], deg_all[:], AF.Sqrt)
            dinv_all = cp.tile([128, NB], F32, tag="dinva")
            nc.vector.reciprocal(dinv_all[:], sq[:])

            # dinv_own[., ib] = dinv_all own half (mask combine)
            t1 = cp.tile([128, HB], F32, tag="t1")
            t2 = cp.tile([128, HB], F32, tag="t2")
            nc.vector.tensor_scalar_mul(t1[:], dinv_all[:, 0:HB], mlo_sb[:])
            nc.vector.tensor_scalar_mul(t2[:], dinv_all[:, HB:NB], mhi_sb[:])
            dinv_own = cp.tile([128, HB], F32, tag="dinvo")
            nc.vector.tensor_tensor(dinv_own[:], t1[:], t2[:], op=ALU.add)
            if dbg:
                nc.sync.dma_start(ddeg[:], deg_sb[:])
                nc.sync.dma_start(ddinv[:], dinv_all[:])
                nc.sync.dma_start(dlw[:], lw_sb[:])
                nc.sync.dma_start(dzx[:], Zx_sb[:])

            # ---------- Xs = dinv_j * X (bf16) ----------
            xs_sb = bigp.tile([128, N], BF16, tag="xssb")
            for jb in range(NB):
                nc.vector.tensor_scalar_mul(
                    xs_sb[:, jb * 128:(jb + 1) * 128],
                    xf_sb[:, jb * 128:(jb + 1) * 128],
                    dinv_all[:, jb:jb + 1],
                )

            # ---------- aggT[d, own-i] = Xs^T @ A_hat^T ----------
            agg_tiles = [psagg.tile([128, 512], F32, tag="agg", name=f"agg{k}")
                         for k in range(4)]
            for jb in range(NB):
                g, jj = jb // 4, jb % 4
                for k in range(4):
                    nc.tensor.matmul(
                        agg_tiles[k][:],
                        xs_sb[:, jb * 128:(jb + 1) * 128],
                        at_g[g][:, jj * HALF + k * 512: jj * HALF + (k + 1) * 512],
                        start=(jb == 0), stop=(jb == NB - 1),
                        skip_group_check=True,
                    )
            aggT_sb = bigp.tile([128, HALF], F32, tag="aggT")
            for k in range(4):
                nc.vector.tensor_copy(aggT_sb[:, k * 512:(k + 1) * 512], agg_tiles[k][:])
            if dbg:
                nc.sync.dma_start(dagg[:], aggT_sb[:, 0:512])
                xs32 = wkp.tile([128, 256], F32, tag="xs32")
                nc.vector.tensor_copy(xs32[:], xs_sb[:, 0:256])
                nc.sync.dma_start(dxs[:], xs32[:])

            # ---------- out = sigmoid(dinv_i * aggT^T @ lw) ----------
            o_sb = bigp.tile([128, HALF], F32, tag="osb")
            for ib in range(HB):
                out_ps = psagg.tile([128, 512], F32, tag="agg")
                nc.tensor.matmul(
                    out_ps[:, 0:128], aggT_sb[:, ib * 128:(ib + 1) * 128], lw_sb[:],
                    start=True, stop=True, skip_group_check=True,
                )
                nc.scalar.activation(
                    o_sb[:, ib * 128:(ib + 1) * 128], out_ps[:, 0:128],
                    AF.Sigmoid, scale=dinv_own[:, ib:ib + 1],
                )
                if ib % 4 == 3:
                    kq = ib // 4
                    nc.sync.dma_start(
                        out[kq * 512:(kq + 1) * 512, :].rearrange("(s p) d -> p s d", p=128),
                        o_sb[:, kq * 512:(kq + 1) * 512].rearrange("p (s d) -> p s d", s=4),
                    )
    nc.compile()
    return nc


PERM = np.concatenate([np.arange(0, 128), np.arange(128, 256),
                       np.arange(384, 512), np.arange(256, 384)])


def kernel(node_embedding, adjacency_matrix, conv_w, conv_b, w_ih, w_hh, b_ih, b_hh):
    if "nc" not in _CACHED:
        _CACHED["nc"] = build_nc()
    nc = _CACHED["nc"]

    X = np.asarray(node_embedding, dtype=np.float32)
    A = np.asarray(adjacency_matrix, dtype=np.float32)
    wih_p = np.asarray(w_ih, dtype=np.float32)[PERM]
    whh_p = np.asarray(w_hh, dtype=np.float32)[PERM]
    bias_p = (np.asarray(b_ih, dtype=np.float32) + np.asarray(b_hh, dtype=np.float32))[PERM]

    common = {
        "cwt": np.ascontiguousarray(np.asarray(conv_w, dtype=np.float32).transpose(2, 1, 0)),
        "wihT": np.ascontiguousarray(wih_p.T),
        "whhT": np.ascontiguousarray(whh_p.T).astype(ml_dtypes.bfloat16),
        "wihn": np.ascontiguousarray(wih_p),
        "bias4": np.ascontiguousarray(bias_p.reshape(4, 128).T),
        "cbb": np.ascontiguousarray(np.broadcast_to(np.asarray(conv_b, np.float32)[None, :], (128, 128))),
        "eyeb": np.eye(128, dtype=ml_dtypes.bfloat16),
    }
    ones = np.ones((128, 1), np.float32)
    zeros = np.zeros((128, 1), np.float32)
    idx = np.arange(HALF)

    in_maps = []
    for c in range(8):
        b, h = c // 2, c % 2
        m = dict(common)
        atf = np.ascontiguousarray(
            A[b, h * HALF:(h + 1) * HALF, :].T, dtype=np.float32
        ).astype(ml_dtypes.bfloat16)
        atf[h * HALF + idx, idx] += 1.0   # bake in A_hat = A + I
        m["atf"] = atf
        m["xf"] = np.ascontiguousarray(X[b])
        m["xdf"] = np.ascontiguousarray(X[b, N - HID:, :])
        m["mlo"] = ones if h == 0 else zeros
        m["mhi"] = zeros if h == 0 else ones
        in_maps.append(m)

    _CACHED["in_maps"] = in_maps
    res = bass_utils.run_bass_kernel_spmd(nc, in_maps, core_ids=list(range(8)))

    outv = np.empty((B, N, HID), np.float32)
    for c in range(8):
        b, h = c // 2, c % 2
        outv[b, h * HALF:(h + 1) * HALF, :] = res.results[c]["out"]
    return outv


# revision 17
# speedup vs baseline: 1.0810x; 1.0810x over previous
import numpy as np
import ml_dtypes

import concourse.bass as bass
import concourse.bacc as bacc
import concourse.mybir as mybir
import concourse.tile as tile
from concourse import bass_utils

F32 = mybir.dt.float32
BF16 = mybir.dt.bfloat16
AF = mybir.ActivationFunctionType
ALU = mybir.AluOpType

B, N, HID = 4, 4096, 128
HALF = N // 2           # own output rows per core
NB = N // 128           # 32 j blocks
HB = HALF // 128        # 16 own i blocks
NGROUP = 8              # stream groups (4 j-blocks each)
NITER = 8               # Jacobi LSTM matmul iterations (plus a zeroth)

_CACHED = {}


def build_nc(dbg=False):
    nc = bacc.Bacc("TRN2", target_bir_lowering=False, debug=False, num_devices=8)
    if dbg:
        ddeg = nc.dram_tensor("ddeg", [128, 16], F32, kind="ExternalOutput")
        ddinv = nc.dram_tensor("ddinv", [128, NB], F32, kind="ExternalOutput")
        dlw = nc.dram_tensor("dlw", [128, 128], F32, kind="ExternalOutput")
        dzx = nc.dram_tensor("dzx", [128, 512], F32, kind="ExternalOutput")
        dagg = nc.dram_tensor("dagg", [128, 512], F32, kind="ExternalOutput")
        dxs = nc.dram_tensor("dxs", [128, 256], F32, kind="ExternalOutput")

    # A_hat^T slice: [4096 j, 2048 own-i] bf16, identity pre-added on host
    atf = nc.dram_tensor("atf", [N, HALF], BF16, kind="ExternalInput")
    xf = nc.dram_tensor("xf", [N, HID], F32, kind="ExternalInput")
    xdf = nc.dram_tensor("xdf", [128, 128], F32, kind="ExternalInput")
    cwt = nc.dram_tensor("cwt", [3, 128, 128], F32, kind="ExternalInput")
    wihT = nc.dram_tensor("wihT", [128, 512], F32, kind="ExternalInput")
    whhT = nc.dram_tensor("whhT", [128, 512], BF16, kind="ExternalInput")
    wihn = nc.dram_tensor("wihn", [512, 128], F32, kind="ExternalInput")
    bias4 = nc.dram_tensor("bias4", [128, 4], F32, kind="ExternalInput")
    cbb = nc.dram_tensor("cbb", [128, 128], F32, kind="ExternalInput")
    eyeb = nc.dram_tensor("eyeb", [128, 128], BF16, kind="ExternalInput")
    mlo = nc.dram_tensor("mlo", [128, 1], F32, kind="ExternalInput")
    mhi = nc.dram_tensor("mhi", [128, 1], F32, kind="ExternalInput")
    out = nc.dram_tensor("out", [HALF, HID], F32, kind="ExternalOutput")

    with tile.TileContext(nc) as tc:
        with (
            tc.tile_pool(name="const", bufs=1) as cp,
            tc.tile_pool(name="big", bufs=1) as bigp,
            tc.tile_pool(name="at", bufs=NGROUP) as atp,
            tc.tile_pool(name="work", bufs=2) as wkp,
            tc.tile_pool(name="ps1", bufs=2, space="PSUM") as ps1,
            tc.tile_pool(name="psdeg", bufs=1, space="PSUM") as psdeg,
            tc.tile_pool(name="psagg", bufs=4, space="PSUM") as psagg,
            tc.tile_pool(name="dram", bufs=1, space="DRAM") as dram,
        ):
            # ---------- stream group 0 first: get A moving immediately ----------
            at_g = []
            t0g = atp.tile([128, 4 * HALF], BF16, tag="at", name="at0")
            nc.sync.dma_start(
                t0g[:].rearrange("p (t c) -> p t c", t=4),
                atf[0:512, :].rearrange("(t p) c -> p t c", p=128),
            )
            at_g.append(t0g)

            # ---------- small constant loads ----------
            wihT_sb = cp.tile([128, 512], F32, tag="wihT")
            whhT_sb = cp.tile([128, 512], BF16, tag="whhT")
            bias4_sb = cp.tile([128, 4], F32, tag="bias4")
            cbb_sb = cp.tile([128, 128], F32, tag="cbb")
            eyeb_sb = cp.tile([128, 128], BF16, tag="eyeb")
            mlo_sb = cp.tile([128, 1], F32, tag="mlo")
            mhi_sb = cp.tile([128, 1], F32, tag="mhi")
            xdf_sb = cp.tile([128, 128], F32, tag="xdf")
            cwt_sb = cp.tile([128, 384], F32, tag="cwt")
            nc.sync.dma_start(wihT_sb[:], wihT[:])
            nc.sync.dma_start(whhT_sb[:], whhT[:])
            nc.sync.dma_start(bias4_sb[:], bias4[:])
            nc.sync.dma_start(cbb_sb[:], cbb[:])
            nc.sync.dma_start(eyeb_sb[:], eyeb[:])
            nc.sync.dma_start(mlo_sb[:], mlo[:])
            nc.sync.dma_start(mhi_sb[:], mhi[:])
            nc.sync.dma_start(xdf_sb[:], xdf[:])
            for k in range(3):
                nc.sync.dma_start(cwt_sb[:, k * 128:(k + 1) * 128], cwt[k])
            wn_sb = cp.tile([128, 512], F32, tag="wn")
            nc.sync.dma_start(
                wn_sb[:].rearrange("p (g d) -> p g d", g=4),
                wihn[:].rearrange("(g p) d -> p g d", p=128),
            )

            ones_sb = cp.tile([128, 1], BF16, tag="ones")
            nc.vector.memset(ones_sb[:], 1.0)

            # ---------- stream remaining A_hat^T groups ----------
            for g in range(1, NGROUP):
                t = atp.tile([128, 4 * HALF], BF16, tag="at", name=f"at{g}")
                nc.sync.dma_start(
                    t[:].rearrange("p (t c) -> p t c", t=4),
                    atf[g * 512:(g + 1) * 512, :].rearrange("(t p) c -> p t c", p=128),
                )
                at_g.append(t)

            # xf after the stream issue (needed later)
            xf_sb = bigp.tile([128, N], F32, tag="xfsb")
            nc.sync.dma_start(
                xf_sb[:].rearrange("p (n d) -> p n d", n=NB),
                xf[:].rearrange("(n p) d -> p n d", p=128),
            )

            # ---------- conv -> dynT ----------
            dfpad = cp.tile([128, 130], F32, tag="dfpad")
            nc.vector.memset(dfpad[:], 0.0)
            nc.vector.tensor_copy(dfpad[:, 1:129], xdf_sb[:])
            dyn_ps = ps1.tile([128, 512], F32, tag="a")
            for k in range(3):
                nc.tensor.matmul(
                    dyn_ps[:, 0:128], dfpad[:, k:k + 128],
                    cwt_sb[:, k * 128:(k + 1) * 128],
                    start=(k == 0), stop=(k == 2),
                )
            dynT_sb = cp.tile([128, 128], F32, tag="dynT")
            nc.vector.tensor_copy(dynT_sb[:], dyn_ps[:, 0:128])

            # S[u, g] = sum_f w_ih_perm[g*128+u, f]
            S_sb = cp.tile([128, 4], F32, tag="S")
            for g4 in range(4):
                nc.vector.reduce_sum(
                    S_sb[:, g4:g4 + 1], wn_sb[:, g4 * 128:(g4 + 1) * 128],
                    axis=mybir.AxisListType.X,
                )

            # ---------- Zx_sb[u, g*128+t] ----------
            zx_ps = ps1.tile([128, 512], F32, tag="a")
            for g4 in range(4):
                nc.tensor.matmul(
                    zx_ps[:, g4 * 128:(g4 + 1) * 128],
                    wihT_sb[:, g4 * 128:(g4 + 1) * 128], dynT_sb[:],
                    start=True, stop=True, skip_group_check=True,
                )
            Zx_sb = bigp.tile([128, 512], F32, tag="Zx")
            for g4 in range(4):
                sl = slice(g4 * 128, (g4 + 1) * 128)
                nc.vector.tensor_scalar_add(
                    Zx_sb[:, sl], zx_ps[:, sl], bias4_sb[:, g4:g4 + 1])
                corr = wkp.tile([128, 128], F32, tag="corr")
                nc.vector.tensor_scalar_mul(corr[:], cbb_sb[:], S_sb[:, g4:g4 + 1])
                nc.vector.tensor_tensor(Zx_sb[:, sl], Zx_sb[:, sl], corr[:], op=ALU.add)

            # ---------- Jacobi LSTM ----------
            hq = cp.tile([128, 129], BF16, tag="hq")   # col0 = 0, cols 1..128 = h_t
            nc.vector.memset(hq[:], 0.0)

            zps = []
            for it in range(NITER + 1):
                zps.append(ps1.tile([128, 512], F32, tag="a", name=f"zp{it}"))

            def lstm_iter(zp):
                ga = wkp.tile([128, 384], F32, tag="ga")
                tg = wkp.tile([128, 128], F32, tag="tg")
                ig = wkp.tile([128, 128], F32, tag="ig")
                cc = wkp.tile([128, 128], F32, tag="cc")
                tcn = wkp.tile([128, 128], F32, tag="tc")
                nc.scalar.activation(ga[:], zp[:, 0:384], AF.Sigmoid)
                nc.scalar.activation(tg[:], zp[:, 384:512], AF.Tanh)
                nc.vector.tensor_tensor(ig[:], ga[:, 0:128], tg[:], op=ALU.mult)
                nc.vector.tensor_tensor_scan(
                    cc[:], ga[:, 128:256], ig[:], 0.0, ALU.mult, ALU.add)
                nc.scalar.activation(tcn[:], cc[:], AF.Tanh)
                nc.vector.tensor_tensor(hq[:, 1:129], ga[:, 256:384], tcn[:], op=ALU.mult)

            # zeroth iteration: gates from Zx only (h = 0)
            nc.vector.tensor_copy(zps[0][:], Zx_sb[:])
            nc.vector.tensor_copy(zps[1][:], Zx_sb[:])
            lstm_iter(zps[0])

            # ---------- deg matmuls interleaved with LSTM iterations ----------
            deg_ps = psdeg.tile([128, 512], F32, tag="deg")
            deg_ps2 = psdeg.tile([128, 512], F32, tag="deg2")

            def deg_slot(k):
                return deg_ps[32 * k:32 * k + 1, :] if k < 3 else deg_ps2[0:1, :]

            def deg_group(g):
                for jj in range(4):
                    jb = g * 4 + jj
                    for k in range(4):
                        nc.tensor.matmul(
                            deg_slot(k),
                            ones_sb[:],
                            at_g[g][:, jj * HALF + k * 512: jj * HALF + (k + 1) * 512],
                            start=(jb == 0), stop=(jb == NB - 1),
                            skip_group_check=True,
                        )

            def full_iter(it):
                # matmuls accumulate onto preloaded Zx in psum bank it%? (ring)
                zp = zps[it]
                for g4 in range(4):
                    nc.tensor.matmul(
                        zp[:, g4 * 128:(g4 + 1) * 128],
                        whhT_sb[:, g4 * 128:(g4 + 1) * 128], hq[:, 0:128],
                        start=False, stop=True, skip_group_check=True,
                    )
                lstm_iter(zp)
                if it + 1 <= NITER:
                    nc.vector.tensor_copy(zps[it + 1][:], Zx_sb[:])

            # interleave: groups 0..4 with iters 1..5, then groups 5..7, then iters 6..8
            for g in range(5):
                deg_group(g)
                full_iter(g + 1)
            for g in range(5, NGROUP):
                deg_group(g)
            for it in range(6, NITER + 1):
                full_iter(it)

            # lw[t, u] = h_t[u] via plain matmul with identity moving
            lw_ps = ps1.tile([128, 512], F32, tag="a")
            nc.tensor.matmul(
                lw_ps[:, 0:128], hq[:, 1:129], eyeb_sb[:],
                start=True, stop=True, skip_group_check=True,
            )
            lw_sb = cp.tile([128, 128], F32, tag="lw")
            nc.vector.tensor_copy(lw_sb[:], lw_ps[:, 0:128])

            # ---------- deg reshape [1,512]x4 -> [128,16] ----------
            # rows of atf/xf are host-reordered: own j-half first, then other.
            deg_row = cp.tile([1, 2048], F32, tag="degrow")
            nc.vector.tensor_copy(deg_row[:, 0:512], deg_slot(0))
            nc.vector.tensor_copy(deg_row[:, 512:1024], deg_slot(1))
            nc.scalar.copy(deg_row[:, 1024:1536], deg_slot(2))
            nc.scalar.copy(deg_row[:, 1536:2048], deg_slot(3))
            deg_lin = dram.tile([1, 2048], F32)
            nc.sync.dma_start(deg_lin[:], deg_row[:])
            deg_sb = cp.tile([128, 16], F32, tag="degsb")
            nc.sync.dma_start(
                deg_sb[:],
                deg_lin[:].rearrange("o (b p) -> (o p) b", p=128),
            )

            # local half: dinv for j-blocks 0..15 (own) available pre-collective
            dinv_all = cp.tile([128, NB], F32, tag="dinva")
            sq = cp.tile([128, NB], F32, tag="sq")
            nc.scalar.activation(sq[:, 0:HB], deg_sb[:], AF.Sqrt)
            nc.vector.reciprocal(dinv_all[:, 0:HB], sq[:, 0:HB])

            # start AllGather of own deg
            cc_in = dram.tile([128, 16], F32)
            cc_out = dram.tile([2, 128, 16], F32)
            nc.gpsimd.dma_start(cc_in[:], deg_sb[:])
            nc.gpsimd.collective_compute(
                "AllGather", ALU.bypass,
                replica_groups=[[0, 1], [2, 3], [4, 5], [6, 7]],
                ins=[cc_in.opt()], outs=[cc_out.opt()],
            )

            # ---------- Xs (own half) + agg over own-j tiles, during cc ----------
            xs_sb = bigp.tile([128, N], BF16, tag="xssb")
            for jb in range(HB):
                nc.vector.tensor_scalar_mul(
                    xs_sb[:, jb * 128:(jb + 1) * 128],
                    xf_sb[:, jb * 128:(jb + 1) * 128],
                    dinv_all[:, jb:jb + 1],
                )
            agg_tiles = [psagg.tile([128, 512], F32, tag="agg", name=f"agg{k}")
                         for k in range(4)]

            def agg_tile(jb):
                g, jj = jb // 4, jb % 4
                for k in range(4):
                    nc.tensor.matmul(
                        agg_tiles[k][:],
                        xs_sb[:, jb * 128:(jb + 1) * 128],
                        at_g[g][:, jj * HALF + k * 512: jj * HALF + (k + 1) * 512],
                        start=(jb == 0), stop=(jb == NB - 1),
                        skip_group_check=True,
                    )

            for jb in range(HB):
                agg_tile(jb)

            # ---------- other half after cc: slot (1-h) via masks ----------
            oth0 = cp.tile([128, 16], F32, tag="oth0")
            oth1 = cp.tile([128, 16], F32, tag="oth1")
            nc.sync.dma_start(oth0[:], cc_out[0])
            nc.sync.dma_start(oth1[:], cc_out[1])
            deg_oth = cp.tile([128, 16], F32, tag="degoth")
            nc.vector.tensor_scalar_mul(oth1[:], oth1[:], mlo_sb[:])
            nc.vector.tensor_scalar_mul(oth0[:], oth0[:], mhi_sb[:])
            nc.vector.tensor_tensor(deg_oth[:], oth0[:], oth1[:], op=ALU.add)
            nc.scalar.activation(sq[:, HB:NB], deg_oth[:], AF.Sqrt)
            nc.vector.reciprocal(dinv_all[:, HB:NB], sq[:, HB:NB])


            if dbg:
                nc.sync.dma_start(ddeg[:], deg_sb[:])
                nc.sync.dma_start(ddinv[:], dinv_all[:])
                nc.sync.dma_start(dlw[:], lw_sb[:])
                nc.sync.dma_start(dzx[:], Zx_sb[:])

            for jb in range(HB, NB):
                nc.vector.tensor_scalar_mul(
                    xs_sb[:, jb * 128:(jb + 1) * 128],
                    xf_sb[:, jb * 128:(jb + 1) * 128],
                    dinv_all[:, jb:jb + 1],
                )
            for jb in range(HB, NB):
                agg_tile(jb)

            aggT_sb = bigp.tile([128, HALF], F32, tag="aggT")
            for k in range(4):
                nc.vector.tensor_copy(aggT_sb[:, k * 512:(k + 1) * 512], agg_tiles[k][:])
            if dbg:
                nc.sync.dma_start(dagg[:], aggT_sb[:, 0:512])
                xs32 = wkp.tile([128, 256], F32, tag="xs32")
                nc.vector.tensor_copy(xs32[:], xs_sb[:, 0:256])
                nc.sync.dma_start(dxs[:], xs32[:])

            # ---------- out = sigmoid(dinv_i * aggT^T @ lw) ----------
            o_sb = bigp.tile([128, HALF], F32, tag="osb")
            for ib in range(HB):
                out_ps = psagg.tile([128, 512], F32, tag="agg")
                nc.tensor.matmul(
                    out_ps[:, 0:128], aggT_sb[:, ib * 128:(ib + 1) * 128], lw_sb[:],
                    start=True, stop=True, skip_group_check=True,
                )
                nc.scalar.activation(
                    o_sb[:, ib * 128:(ib + 1) * 128], out_ps[:, 0:128],
                    AF.Sigmoid, scale=dinv_all[:, ib:ib + 1],
                )
                if ib % 4 == 3:
                    kq = ib // 4
                    nc.sync.dma_start(
                        out[kq * 512:(kq + 1) * 512, :].rearrange("(s p) d -> p s d", p=128),
                        o_sb[:, kq * 512:(kq + 1) * 512].rearrange("p (s d) -> p s d", s=4),
                    )
    nc.compile()
    return nc


PERM = np.concatenate([np.arange(0, 128), np.arange(128, 256),
                       np.arange(384, 512), np.arange(256, 384)])


def kernel(node_embedding, adjacency_matrix, conv_w, conv_b, w_ih, w_hh, b_ih, b_hh):
    if "nc" not in _CACHED:
        _CACHED["nc"] = build_nc()
    nc = _CACHED["nc"]

    X = np.asarray(node_embedding, dtype=np.float32)
    A = np.asarray(adjacency_matrix, dtype=np.float32)
    wih_p = np.asarray(w_ih, dtype=np.float32)[PERM]
    whh_p = np.asarray(w_hh, dtype=np.float32)[PERM]
    bias_p = (np.asarray(b_ih, dtype=np.float32) + np.asarray(b_hh, dtype=np.float32))[PERM]

    common = {
        "cwt": np.ascontiguousarray(np.asarray(conv_w, dtype=np.float32).transpose(2, 1, 0)),
        "wihT": np.ascontiguousarray(wih_p.T),
        "whhT": np.ascontiguousarray(whh_p.T).astype(ml_dtypes.bfloat16),
        "wihn": np.ascontiguousarray(wih_p),
        "bias4": np.ascontiguousarray(bias_p.reshape(4, 128).T),
        "cbb": np.ascontiguousarray(np.broadcast_to(np.asarray(conv_b, np.float32)[None, :], (128, 128))),
        "eyeb": np.eye(128, dtype=ml_dtypes.bfloat16),
    }
    ones = np.ones((128, 1), np.float32)
    zeros = np.zeros((128, 1), np.float32)
    idx = np.arange(HALF)

    in_maps = []
    for c in range(8):
        b, h = c // 2, c % 2
        own = slice(h * HALF, (h + 1) * HALF)
        oth = slice((1 - h) * HALF, (2 - h) * HALF)
        m = dict(common)
        sT = A[b, own, :].T   # [4096 j, 2048 own-i]
        atf = np.concatenate([sT[own], sT[oth]])  # own j-half first
        atf = np.ascontiguousarray(atf).astype(ml_dtypes.bfloat16)
        atf[idx, idx] += 1.0   # bake in A_hat = A + I (diag is in the top block)
        m["atf"] = atf
        m["xf"] = np.ascontiguousarray(np.concatenate([X[b, own], X[b, oth]]))
        m["xdf"] = np.ascontiguousarray(X[b, N - HID:, :])
        m["mlo"] = ones if h == 0 else zeros
        m["mhi"] = zeros if h == 0 else ones
        in_maps.append(m)

    _CACHED["in_maps"] = in_maps
    res = bass_utils.run_bass_kernel_spmd(nc, in_maps, core_ids=list(range(8)))

    outv = np.empty((B, N, HID), np.float32)
    for c in range(8):
        b, h = c // 2, c % 2
        outv[b, h * HALF:(h + 1) * HALF, :] = res.results[c]["out"]
    return outv


# revision 20
# speedup vs baseline: 1.1731x; 1.0852x over previous
import numpy as np
import ml_dtypes

import concourse.bass as bass
import concourse.bacc as bacc
import concourse.mybir as mybir
import concourse.tile as tile
from concourse import bass_utils

F32 = mybir.dt.float32
BF16 = mybir.dt.bfloat16
AF = mybir.ActivationFunctionType
ALU = mybir.AluOpType

B, N, HID = 4, 4096, 128
HALF = N // 2           # own output rows per core
NB = N // 128           # 32 j blocks
HB = HALF // 128        # 16 own i blocks
NGROUP = 8              # stream groups (4 j-blocks each)
NITER = 6               # Jacobi LSTM matmul iterations (plus a zeroth)

_CACHED = {}


def build_nc(dbg=False):
    nc = bacc.Bacc("TRN2", target_bir_lowering=False, debug=False, num_devices=8)
    if dbg:
        ddeg = nc.dram_tensor("ddeg", [128, 16], F32, kind="ExternalOutput")
        ddinv = nc.dram_tensor("ddinv", [128, NB], F32, kind="ExternalOutput")
        dlw = nc.dram_tensor("dlw", [128, 128], F32, kind="ExternalOutput")
        dzx = nc.dram_tensor("dzx", [128, 512], F32, kind="ExternalOutput")
        dagg = nc.dram_tensor("dagg", [128, 512], F32, kind="ExternalOutput")
        dxs = nc.dram_tensor("dxs", [128, 256], F32, kind="ExternalOutput")

    # A_hat^T slice: [4096 j, 2048 own-i] bf16, identity pre-added on host
    atf = nc.dram_tensor("atf", [N, HALF], BF16, kind="ExternalInput")
    xf = nc.dram_tensor("xf", [N, HID], F32, kind="ExternalInput")
    xdf = nc.dram_tensor("xdf", [128, 128], F32, kind="ExternalInput")
    cwt = nc.dram_tensor("cwt", [3, 128, 128], F32, kind="ExternalInput")
    wihT = nc.dram_tensor("wihT", [128, 512], F32, kind="ExternalInput")
    whhT = nc.dram_tensor("whhT", [128, 512], BF16, kind="ExternalInput")
    wihn = nc.dram_tensor("wihn", [512, 128], F32, kind="ExternalInput")
    bias4 = nc.dram_tensor("bias4", [128, 4], F32, kind="ExternalInput")
    cbb = nc.dram_tensor("cbb", [128, 128], F32, kind="ExternalInput")
    eyeb = nc.dram_tensor("eyeb", [128, 128], BF16, kind="ExternalInput")
    mlo = nc.dram_tensor("mlo", [128, 1], F32, kind="ExternalInput")
    mhi = nc.dram_tensor("mhi", [128, 1], F32, kind="ExternalInput")
    out = nc.dram_tensor("out", [HALF, HID], F32, kind="ExternalOutput")

    with tile.TileContext(nc) as tc:
        with (
            tc.tile_pool(name="const", bufs=1) as cp,
            tc.tile_pool(name="big", bufs=1) as bigp,
            tc.tile_pool(name="at", bufs=NGROUP) as atp,
            tc.tile_pool(name="work", bufs=2) as wkp,
            tc.tile_pool(name="ps1", bufs=2, space="PSUM") as ps1,
            tc.tile_pool(name="psdeg", bufs=1, space="PSUM") as psdeg,
            tc.tile_pool(name="psagg", bufs=4, space="PSUM") as psagg,
            tc.tile_pool(name="dram", bufs=1, space="DRAM") as dram,
        ):
            # ---------- stream group 0 first: get A moving immediately ----------
            at_g = []
            t0g = atp.tile([128, 4 * HALF], BF16, tag="at", name="at0")
            nc.sync.dma_start(
                t0g[:].rearrange("p (t c) -> p t c", t=4),
                atf[0:512, :].rearrange("(t p) c -> p t c", p=128),
            )
            at_g.append(t0g)

            # ---------- small constant loads ----------
            wihT_sb = cp.tile([128, 512], F32, tag="wihT")
            whhT_sb = cp.tile([128, 512], BF16, tag="whhT")
            bias4_sb = cp.tile([128, 4], F32, tag="bias4")
            cbb_sb = cp.tile([128, 128], F32, tag="cbb")
            eyeb_sb = cp.tile([128, 128], BF16, tag="eyeb")
            mlo_sb = cp.tile([128, 1], F32, tag="mlo")
            mhi_sb = cp.tile([128, 1], F32, tag="mhi")
            xdf_sb = cp.tile([128, 128], F32, tag="xdf")
            cwt_sb = cp.tile([128, 384], F32, tag="cwt")
            nc.sync.dma_start(wihT_sb[:], wihT[:])
            nc.sync.dma_start(whhT_sb[:], whhT[:])
            nc.sync.dma_start(bias4_sb[:], bias4[:])
            nc.sync.dma_start(cbb_sb[:], cbb[:])
            nc.sync.dma_start(eyeb_sb[:], eyeb[:])
            nc.sync.dma_start(mlo_sb[:], mlo[:])
            nc.sync.dma_start(mhi_sb[:], mhi[:])
            nc.sync.dma_start(xdf_sb[:], xdf[:])
            for k in range(3):
                nc.sync.dma_start(cwt_sb[:, k * 128:(k + 1) * 128], cwt[k])
            wn_sb = cp.tile([128, 512], F32, tag="wn")
            nc.sync.dma_start(
                wn_sb[:].rearrange("p (g d) -> p g d", g=4),
                wihn[:].rearrange("(g p) d -> p g d", p=128),
            )

            ones_sb = cp.tile([128, 1], BF16, tag="ones")
            nc.vector.memset(ones_sb[:], 1.0)

            # ---------- stream remaining A_hat^T groups ----------
            for g in range(1, NGROUP):
                t = atp.tile([128, 4 * HALF], BF16, tag="at", name=f"at{g}")
                nc.sync.dma_start(
                    t[:].rearrange("p (t c) -> p t c", t=4),
                    atf[g * 512:(g + 1) * 512, :].rearrange("(t p) c -> p t c", p=128),
                )
                at_g.append(t)

            # xf after the stream issue (needed later)
            xf_sb = bigp.tile([128, N], F32, tag="xfsb")
            nc.sync.dma_start(
                xf_sb[:].rearrange("p (n d) -> p n d", n=NB),
                xf[:].rearrange("(n p) d -> p n d", p=128),
            )

            # ---------- conv -> dynT ----------
            dfpad = cp.tile([128, 130], F32, tag="dfpad")
            nc.vector.memset(dfpad[:], 0.0)
            nc.vector.tensor_copy(dfpad[:, 1:129], xdf_sb[:])
            dyn_ps = ps1.tile([128, 512], F32, tag="a")
            for k in range(3):
                nc.tensor.matmul(
                    dyn_ps[:, 0:128], dfpad[:, k:k + 128],
                    cwt_sb[:, k * 128:(k + 1) * 128],
                    start=(k == 0), stop=(k == 2),
                )
            dynT_sb = cp.tile([128, 128], F32, tag="dynT")
            nc.vector.tensor_copy(dynT_sb[:], dyn_ps[:, 0:128])

            # S[u, g] = sum_f w_ih_perm[g*128+u, f]
            S_sb = cp.tile([128, 4], F32, tag="S")
            for g4 in range(4):
                nc.vector.reduce_sum(
                    S_sb[:, g4:g4 + 1], wn_sb[:, g4 * 128:(g4 + 1) * 128],
                    axis=mybir.AxisListType.X,
                )

            # ---------- Zx_sb[u, g*128+t] ----------
            zx_ps = ps1.tile([128, 512], F32, tag="a")
            for g4 in range(4):
                nc.tensor.matmul(
                    zx_ps[:, g4 * 128:(g4 + 1) * 128],
                    wihT_sb[:, g4 * 128:(g4 + 1) * 128], dynT_sb[:],
                    start=True, stop=True, skip_group_check=True,
                )
            Zx_sb = bigp.tile([128, 512], F32, tag="Zx")
            for g4 in range(4):
                sl = slice(g4 * 128, (g4 + 1) * 128)
                nc.vector.tensor_scalar_add(
                    Zx_sb[:, sl], zx_ps[:, sl], bias4_sb[:, g4:g4 + 1])
                corr = wkp.tile([128, 128], F32, tag="corr")
                nc.vector.tensor_scalar_mul(corr[:], cbb_sb[:], S_sb[:, g4:g4 + 1])
                nc.vector.tensor_tensor(Zx_sb[:, sl], Zx_sb[:, sl], corr[:], op=ALU.add)

            # ---------- Jacobi LSTM ----------
            hq = cp.tile([128, 129], BF16, tag="hq")   # col0 = 0, cols 1..128 = h_t
            nc.vector.memset(hq[:], 0.0)

            zps = []
            for it in range(NITER + 1):
                zps.append(ps1.tile([128, 512], F32, tag="a", name=f"zp{it}"))

            def lstm_iter(zp):
                ga = wkp.tile([128, 384], F32, tag="ga")
                tg = wkp.tile([128, 128], F32, tag="tg")
                ig = wkp.tile([128, 128], F32, tag="ig")
                cc = wkp.tile([128, 128], F32, tag="cc")
                tcn = wkp.tile([128, 128], F32, tag="tc")
                nc.scalar.activation(ga[:], zp[:, 0:384], AF.Sigmoid)
                nc.scalar.activation(tg[:], zp[:, 384:512], AF.Tanh)
                nc.vector.tensor_tensor(ig[:], ga[:, 0:128], tg[:], op=ALU.mult)
                nc.vector.tensor_tensor_scan(
                    cc[:], ga[:, 128:256], ig[:], 0.0, ALU.mult, ALU.add)
                nc.scalar.activation(tcn[:], cc[:], AF.Tanh)
                nc.vector.tensor_tensor(hq[:, 1:129], ga[:, 256:384], tcn[:], op=ALU.mult)

            # zeroth iteration: gates from Zx only (h = 0)
            nc.vector.tensor_copy(zps[0][:], Zx_sb[:])
            nc.vector.tensor_copy(zps[1][:], Zx_sb[:])
            lstm_iter(zps[0])

            # ---------- deg matmuls interleaved with LSTM iterations ----------
            deg_ps = psdeg.tile([128, 512], F32, tag="deg")
            deg_ps2 = psdeg.tile([128, 512], F32, tag="deg2")

            def deg_slot(k):
                return deg_ps[32 * k:32 * k + 1, :] if k < 3 else deg_ps2[0:1, :]

            def deg_group(g):
                for jj in range(4):
                    jb = g * 4 + jj
                    for k in range(4):
                        nc.tensor.matmul(
                            deg_slot(k),
                            ones_sb[:],
                            at_g[g][:, jj * HALF + k * 512: jj * HALF + (k + 1) * 512],
                            start=(jb == 0), stop=(jb == NB - 1),
                            skip_group_check=True,
                        )

            def full_iter(it):
                # matmuls accumulate onto preloaded Zx in psum bank it%? (ring)
                zp = zps[it]
                for g4 in range(4):
                    nc.tensor.matmul(
                        zp[:, g4 * 128:(g4 + 1) * 128],
                        whhT_sb[:, g4 * 128:(g4 + 1) * 128], hq[:, 0:128],
                        start=False, stop=True, skip_group_check=True,
                    )
                lstm_iter(zp)
                if it + 1 <= NITER:
                    nc.vector.tensor_copy(zps[it + 1][:], Zx_sb[:])

            # interleave: groups 0..5 with iters 1..6, then groups 6..7
            for g in range(NITER):
                deg_group(g)
                full_iter(g + 1)
            for g in range(NITER, NGROUP):
                deg_group(g)

            # ---------- deg reshape + collective launch, ASAP after deg stop ----------
            # rows of atf/xf are host-reordered: own j-half first, then other.
            deg_row = cp.tile([1, 2048], F32, tag="degrow")
            nc.vector.tensor_copy(deg_row[:, 0:512], deg_slot(0))
            nc.vector.tensor_copy(deg_row[:, 512:1024], deg_slot(1))
            nc.vector.tensor_copy(deg_row[:, 1024:1536], deg_slot(2))
            nc.vector.tensor_copy(deg_row[:, 1536:2048], deg_slot(3))
            deg_lin = dram.tile([1, 2048], F32)
            nc.sync.dma_start(deg_lin[:], deg_row[:])
            deg_sb = cp.tile([128, 16], F32, tag="degsb")
            nc.sync.dma_start(
                deg_sb[:],
                deg_lin[:].rearrange("o (b p) -> (o p) b", p=128),
            )
            cc_in = dram.tile([128, 16], F32)
            cc_out = dram.tile([2, 128, 16], F32)
            nc.gpsimd.dma_start(cc_in[:], deg_sb[:])
            nc.gpsimd.collective_compute(
                "AllGather", ALU.bypass,
                replica_groups=[[0, 1], [2, 3], [4, 5], [6, 7]],
                ins=[cc_in.opt()], outs=[cc_out.opt()],
            )

            # local half: dinv for j-blocks 0..15 (own) available pre-collective
            dinv_all = cp.tile([128, NB], F32, tag="dinva")
            sq = cp.tile([128, NB], F32, tag="sq")
            nc.scalar.activation(sq[:, 0:HB], deg_sb[:], AF.Sqrt)
            nc.vector.reciprocal(dinv_all[:, 0:HB], sq[:, 0:HB])

            # lw[t, u] = h_t[u] via plain matmul with identity moving
            lw_ps = ps1.tile([128, 512], F32, tag="a")
            nc.tensor.matmul(
                lw_ps[:, 0:128], hq[:, 1:129], eyeb_sb[:],
                start=True, stop=True, skip_group_check=True,
            )
            lw_sb = cp.tile([128, 128], F32, tag="lw")
            nc.vector.tensor_copy(lw_sb[:], lw_ps[:, 0:128])

            # ---------- Xs (own half) + agg over own-j tiles, during cc ----------
            xs_sb = bigp.tile([128, N], BF16, tag="xssb")
            for jb in range(HB):
                nc.vector.tensor_scalar_mul(
                    xs_sb[:, jb * 128:(jb + 1) * 128],
                    xf_sb[:, jb * 128:(jb + 1) * 128],
                    dinv_all[:, jb:jb + 1],
                )
            agg_tiles = [psagg.tile([128, 512], F32, tag="agg", name=f"agg{k}")
                         for k in range(4)]

            def agg_tile(jb):
                g, jj = jb // 4, jb % 4
                for k in range(4):
                    nc.tensor.matmul(
                        agg_tiles[k][:],
                        xs_sb[:, jb * 128:(jb + 1) * 128],
                        at_g[g][:, jj * HALF + k * 512: jj * HALF + (k + 1) * 512],
                        start=(jb == 0), stop=(jb == NB - 1),
                        skip_group_check=True,
                    )

            for jb in range(HB):
                agg_tile(jb)

            # ---------- other half after cc: slot (1-h) via masks ----------
            oth0 = cp.tile([128, 16], F32, tag="oth0")
            oth1 = cp.tile([128, 16], F32, tag="oth1")
            nc.sync.dma_start(oth0[:], cc_out[0])
            nc.sync.dma_start(oth1[:], cc_out[1])
            deg_oth = cp.tile([128, 16], F32, tag="degoth")
            nc.vector.tensor_scalar_mul(oth1[:], oth1[:], mlo_sb[:])
            nc.vector.tensor_scalar_mul(oth0[:], oth0[:], mhi_sb[:])
            nc.vector.tensor_tensor(deg_oth[:], oth0[:], oth1[:], op=ALU.add)
            nc.scalar.activation(sq[:, HB:NB], deg_oth[:], AF.Sqrt)
            nc.vector.reciprocal(dinv_all[:, HB:NB], sq[:, HB:NB])


            if dbg:
                nc.sync.dma_start(ddeg[:], deg_sb[:])
                nc.sync.dma_start(ddinv[:], dinv_all[:])
                nc.sync.dma_start(dlw[:], lw_sb[:])
                nc.sync.dma_start(dzx[:], Zx_sb[:])

            for jb in range(HB, NB):
                nc.vector.tensor_scalar_mul(
                    xs_sb[:, jb * 128:(jb + 1) * 128],
                    xf_sb[:, jb * 128:(jb + 1) * 128],
                    dinv_all[:, jb:jb + 1],
                )
            for jb in range(HB, NB):
                agg_tile(jb)

            # ---------- copy chunks + out = sigmoid(dinv_i * aggT^T @ lw) ----------
            aggT_sb = bigp.tile([128, HALF], F32, tag="aggT")
            o_sb = bigp.tile([128, HALF], F32, tag="osb")
            for kq in range(4):
                nc.vector.tensor_copy(
                    aggT_sb[:, kq * 512:(kq + 1) * 512], agg_tiles[kq][:])
                for ib in range(kq * 4, kq * 4 + 4):
                    out_ps = ps1.tile([128, 512], F32, tag="a")
                    nc.tensor.matmul(
                        out_ps[:, 0:128], aggT_sb[:, ib * 128:(ib + 1) * 128], lw_sb[:],
                        start=True, stop=True, skip_group_check=True,
                    )
                    nc.scalar.activation(
                        o_sb[:, ib * 128:(ib + 1) * 128], out_ps[:, 0:128],
                        AF.Sigmoid, scale=dinv_all[:, ib:ib + 1],
                    )
                nc.sync.dma_start(
                    out[kq * 512:(kq + 1) * 512, :].rearrange("(s p) d -> p s d", p=128),
                    o_sb[:, kq * 512:(kq + 1) * 512].rearrange("p (s d) -> p s d", s=4),
                )
            if dbg:
                nc.sync.dma_start(dagg[:], aggT_sb[:, 0:512])
                xs32 = wkp.tile([128, 256], F32, tag="xs32")
                nc.vector.tensor_copy(xs32[:], xs_sb[:, 0:256])
                nc.sync.dma_start(dxs[:], xs32[:])
    nc.compile()
    return nc


PERM = np.concatenate([np.arange(0, 128), np.arange(128, 256),
                       np.arange(384, 512), np.arange(256, 384)])


def kernel(node_embedding, adjacency_matrix, conv_w, conv_b, w_ih, w_hh, b_ih, b_hh):
    if "nc" not in _CACHED:
        _CACHED["nc"] = build_nc()
    nc = _CACHED["nc"]

    X = np.asarray(node_embedding, dtype=np.float32)
    A = np.asarray(adjacency_matrix, dtype=np.float32)
    wih_p = np.asarray(w_ih, dtype=np.float32)[PERM]
    whh_p = np.asarray(w_hh, dtype=np.float32)[PERM]
    bias_p = (np.asarray(b_ih, dtype=np.float32) + np.asarray(b_hh, dtype=np.float32))[PERM]

    common = {
        "cwt": np.ascontiguousarray(np.asarray(conv_w, dtype=np.float32).transpose(2, 1, 0)),
        "wihT": np.ascontiguousarray(wih_p.T),
        "whhT": np.ascontiguousarray(whh_p.T).astype(ml_dtypes.bfloat16),
        "wihn": np.ascontiguousarray(wih_p),
        "bias4": np.ascontiguousarray(bias_p.reshape(4, 128).T),
        "cbb": np.ascontiguousarray(np.broadcast_to(np.asarray(conv_b, np.float32)[None, :], (128, 128))),
        "eyeb": np.eye(128, dtype=ml_dtypes.bfloat16),
    }
    ones = np.ones((128, 1), np.float32)
    zeros = np.zeros((128, 1), np.float32)
    idx = np.arange(HALF)

    in_maps = []
    for c in range(8):
        b, h = c // 2, c % 2
        own = slice(h * HALF, (h + 1) * HALF)
        oth = slice((1 - h) * HALF, (2 - h) * HALF)
        m = dict(common)
        sT = A[b, own, :].T   # [4096 j, 2048 own-i]
        atf = np.concatenate([sT[own], sT[oth]])  # own j-half first
        atf = np.ascontiguousarray(atf).astype(ml_dtypes.bfloat16)
        atf[idx, idx] += 1.0   # bake in A_hat = A + I (diag is in the top block)
        m["atf"] = atf
        m["xf"] = np.ascontiguousarray(np.concatenate([X[b, own], X[b, oth]]))
        m["xdf"] = np.ascontiguousarray(X[b, N - HID:, :])
        m["mlo"] = ones if h == 0 else zeros
        m["mhi"] = zeros if h == 0 else ones
        in_maps.append(m)

    _CACHED["in_maps"] = in_maps
    res = bass_utils.run_bass_kernel_spmd(nc, in_maps, core_ids=list(range(8)))

    outv = np.empty((B, N, HID), np.float32)
    for c in range(8):
        b, h = c // 2, c % 2
        outv[b, h * HALF:(h + 1) * HALF, :] = res.results[c]["out"]
    return outv
